# revision 16
# baseline (speedup 1.0000x reference)
"""BeforeRNNAttention pooling kernel for 8 TRN2 NeuronCores.

Reference computation (per batch element b):
    e_dec[b]   = si_1[b, :] @ Wd + bias          (Wd = W[:, :DHS])
    e_enc[s,b] = h[s, b, :] @ We                 (We = W[:, DHS:])
    energy     = relu(e_dec + e_enc)             [S, B]
    att        = softmax(energy, axis=s)
    out[b, :]  = sum_s att[s, b] * h[s, b, :]

Sharding: data-parallel over batch (8 batch elements per core). Each core
reads its h shard from HBM exactly once (memory-roofline bound):
  - DVE: one fused tensor_tensor_reduce per [128s, 256e] block computes
    e_dec[b] + sum_e h*We (single pass over h).
  - ACT: relu + exp (batched per 8-tile group), denominator via accum_out.
  - PE : weighted sum with p (exp weights) as 1-column stationary operand
    and h streaming as rhs -> out[1, 256e] accumulated in PSUM.
  - relu >= 0 bounds energies in [0, ~6], so exp without max-subtraction
    is exact-safe in fp32 (matches softmax identically in exact math).
"""

import numpy as np

ESL, B, EHS, DHS = 4096, 64, 256, 256
N_CORES = 8
B_LOC = B // N_CORES
P = 128

_PROG_CACHE = {}


def build_program(
    b_loc=B_LOC, seq=ESL, ehs=EHS, dhs=DHS, g_tiles=8, h_bufs=6, with_tick=False
):
    """Build the single-core SPMD Bass/Tile program.

    with_tick adds a [1,1] "tick" input copied to a "tock" output, so a
    test harness can chain executions serially for timing. The graded
    kernel path never sets it.
    """
    import concourse.bacc as bacc
    import concourse.bass as bass
    import concourse.mybir as mybir
    import concourse.tile as tile

    f32 = mybir.dt.float32
    f32r = mybir.dt.float32r
    AF = mybir.ActivationFunctionType
    ALU = mybir.AluOpType

    n_tiles = seq // P
    n_groups = n_tiles // g_tiles
    assert n_groups * g_tiles == n_tiles
    assert dhs == 2 * P and ehs == 2 * P

    nc = bacc.Bacc(None)
    # h arrives host-pre-rounded to fp32r (11 mantissa bits) so the PE can run
    # the weighted-sum matmuls at full rate (fp32 matmul is 4 cycles/row).
    h_d = nc.declare_dram_parameter("h", [b_loc, seq, ehs], f32r, isOutput=False)
    si_d = nc.declare_dram_parameter("si1t", [dhs + 1, b_loc], f32, isOutput=False)
    wd_d = nc.declare_dram_parameter("wd", [dhs + 1, 1], f32, isOutput=False)
    we_d = nc.declare_dram_parameter("web", [P, ehs], f32, isOutput=False)
    oc_d = nc.declare_dram_parameter("ones_col", [P, 1], f32, isOutput=False)
    or_d = nc.declare_dram_parameter("ones_row", [1, P], f32, isOutput=False)
    out_d = nc.declare_dram_parameter("out", [b_loc, ehs], f32, isOutput=True)
    tick_d = tock_d = None
    if with_tick:
        tick_d = nc.declare_dram_parameter("tick", [1, 1], f32, isOutput=False)
        tock_d = nc.declare_dram_parameter("tock", [1, 1], f32, isOutput=True)

    with tile.TileContext(nc) as tc:
        with (
            tc.tile_pool(name="const", bufs=1) as cpool,
            tc.tile_pool(name="hdat", bufs=h_bufs) as hpool,
            tc.tile_pool(name="work", bufs=2) as wpool,
            tc.tile_pool(name="scratch", bufs=1) as jpool,
            tc.tile_pool(name="pctx", bufs=2, space=bass.MemorySpace.PSUM) as ctxpool,
            tc.tile_pool(name="pden", bufs=2, space=bass.MemorySpace.PSUM) as denpool,
            tc.tile_pool(name="psetup", bufs=1, space=bass.MemorySpace.PSUM) as spool,
        ):
            # ---- constants / setup ----
            web = cpool.tile([P, ehs], f32)
            nc.scalar.dma_start(web[:], we_d[:])
            onc = cpool.tile([P, 1], f32)
            nc.scalar.dma_start(onc[:], oc_d[:])
            onr = cpool.tile([1, P], f32)
            nc.scalar.dma_start(onr[:], or_d[:])

            si0 = cpool.tile([P, b_loc], f32)
            nc.scalar.dma_start(si0[:], si_d[0:P, :])
            si1 = cpool.tile([P, b_loc], f32)
            nc.scalar.dma_start(si1[:], si_d[P : 2 * P, :])
            si2 = cpool.tile([1, b_loc], f32)
            nc.scalar.dma_start(si2[:], si_d[2 * P : 2 * P + 1, :])
            wd0 = cpool.tile([P, 1], f32)
            nc.scalar.dma_start(wd0[:], wd_d[0:P, :])
            wd1 = cpool.tile([P, 1], f32)
            nc.scalar.dma_start(wd1[:], wd_d[P : 2 * P, :])
            wd2 = cpool.tile([1, 1], f32)
            nc.scalar.dma_start(wd2[:], wd_d[2 * P : 2 * P + 1, :])

            # e_dec[1, b] = sum_d wd[d] * si1t[d, b]  (+ bias via appended row)
            edec_ps = spool.tile([1, b_loc], f32)
            nc.tensor.matmul(edec_ps[:], wd0[:], si0[:], start=True, stop=False)
            nc.tensor.matmul(edec_ps[:], wd1[:], si1[:], start=False, stop=False)
            nc.tensor.matmul(edec_ps[:], wd2[:], si2[:], start=False, stop=True)
            edec_sb = cpool.tile([1, b_loc], f32)
            nc.vector.tensor_copy(edec_sb[:], edec_ps[:])
            # broadcast over 128 partitions: ones[1,128].T @ edec[1,b] -> [128,b]
            edecb_ps = spool.tile([P, b_loc], f32)
            nc.tensor.matmul(edecb_ps[:], onr[:], edec_sb[:], start=True, stop=True)
            edecb = cpool.tile([P, b_loc], f32)
            nc.vector.tensor_copy(edecb[:], edecb_ps[:])

            junk = jpool.tile([P, ehs], f32)

            # ---- main loop over local batch elements ----
            for b in range(b_loc):
                # partition p holds g_tiles consecutive s-rows -> the DMA source
                # for each partition is one contiguous 8KB chunk (order over s
                # is irrelevant: softmax/weighted-sum reduce over all of s)
                h_b = h_d[b].rearrange("(q p g) e -> q p (g e)", g=g_tiles, p=P)
                dcol = wpool.tile([P, n_groups], f32, tag="dcol")
                ctx_ps = ctxpool.tile([1, ehs], f32, tag="ctx")
                for q in range(n_groups):
                    hg = hpool.tile([P, g_tiles * ehs], f32r, tag="hg")
                    nc.sync.dma_start(hg[:], h_b[q])
                    e_g = wpool.tile([P, g_tiles], f32, tag="e_g")
                    for g in range(g_tiles):
                        # e_g[:, g] = sum_e h[s, e] * We[e]  (fused mul+reduce,
                        # standard TensorScalarPtr opcode)
                        nc.vector.scalar_tensor_tensor(
                            out=junk[:],
                            in0=hg[:, g * ehs : (g + 1) * ehs].bitcast(f32),
                            scalar=1.0,
                            in1=web[:],
                            op0=ALU.mult,
                            op1=ALU.mult,
                            accum_out=e_g[:, g : g + 1],
                        )
                    # relu(e_enc + e_dec[b]): e_dec enters as the ACT bias
                    ptmp = wpool.tile([P, g_tiles], f32, tag="ptmp")
                    nc.scalar.activation(
                        ptmp[:], e_g[:], AF.Relu, bias=edecb[:, b : b + 1]
                    )
                    p_g = wpool.tile([P, g_tiles], f32r, tag="p_g")
                    nc.scalar.activation(
                        p_g[:], ptmp[:], AF.Exp, accum_out=dcol[:, q : q + 1]
                    )
                    for g in range(g_tiles):
                        t = q * g_tiles + g
                        nc.tensor.matmul(
                            ctx_ps[:],
                            p_g[:, g : g + 1],
                            hg[:, g * ehs : (g + 1) * ehs],
                            start=(t == 0),
                            stop=(t == n_tiles - 1),
                        )
                # denominator = sum over all partitions+groups of exp sums
                dsum = wpool.tile([P, 1], f32, tag="dsum")
                nc.vector.tensor_reduce(
                    dsum[:], dcol[:], axis=mybir.AxisListType.X, op=ALU.add
                )
                den_ps = denpool.tile([1, 1], f32, tag="den")
                nc.tensor.matmul(den_ps[:], dsum[:], onc[:], start=True, stop=True)
                rcp = wpool.tile([1, 1], f32, tag="rcp")
                nc.vector.reciprocal(rcp[:], den_ps[:])
                orow = wpool.tile([1, ehs], f32, tag="orow")
                nc.vector.tensor_scalar_mul(orow[:], ctx_ps[:], rcp[:])
                nc.scalar.dma_start(out_d[b : b + 1, :], orow[:])

            if with_tick:
                tick_sb = cpool.tile([1, 1], f32)
                nc.scalar.dma_start(tick_sb[:], tick_d[:])
                tock_sb = cpool.tile([1, 1], f32)
                # depend on the last batch element's result so the tock DMA
                # lands after the real work
                nc.vector.tensor_scalar_mul(tock_sb[:], tick_sb[:], rcp[:])
                nc.scalar.dma_start(tock_d[:], tock_sb[:])

    nc.compile()
    return nc


def round_to_f32r(x):
    """Round f32 to fp32r precision (11 explicit mantissa bits, RNE)."""
    u = x.view(np.uint32)
    shift = 12  # 23 - 11
    bias = ((u >> shift) & 1).astype(np.uint32) + np.uint32((1 << (shift - 1)) - 1)
    u = (u + bias) & np.uint32(~((1 << shift) - 1) & 0xFFFFFFFF)
    return u.view(np.float32)


def make_in_maps(si_1, h, W, bias, b_loc=B_LOC, n_cores=N_CORES):
    """Shard the full inputs into per-core input maps."""
    si_1 = np.asarray(si_1, dtype=np.float32)
    h = round_to_f32r(np.ascontiguousarray(np.asarray(h, dtype=np.float32)))
    W = np.asarray(W, dtype=np.float32)
    bias = np.asarray(bias, dtype=np.float32)
    dhs = si_1.shape[-1]

    wd_ext = np.concatenate([W[0, :dhs], bias]).reshape(dhs + 1, 1)
    wd_ext = np.ascontiguousarray(wd_ext, dtype=np.float32)
    web = np.ascontiguousarray(np.tile(W[0:1, dhs:], (P, 1)), dtype=np.float32)
    ones_col = np.ones((P, 1), dtype=np.float32)
    ones_row = np.ones((1, P), dtype=np.float32)

    in_maps = []
    for c in range(n_cores):
        sl = slice(c * b_loc, (c + 1) * b_loc)
        h_c = np.ascontiguousarray(h[:, sl, :].transpose(1, 0, 2))
        si_c = np.concatenate(
            [si_1[0, sl, :].T, np.ones((1, b_loc), np.float32)], axis=0
        )
        in_maps.append(
            {
                "h": h_c,
                "si1t": np.ascontiguousarray(si_c, dtype=np.float32),
                "wd": wd_ext,
                "web": web,
                "ones_col": ones_col,
                "ones_row": ones_row,
            }
        )
    return in_maps


def _get_prog():
    key = (B_LOC, ESL, EHS, DHS)
    if key not in _PROG_CACHE:
        _PROG_CACHE[key] = build_program()
    return _PROG_CACHE[key]


def kernel(si_1, h, W, b):
    from concourse.bass_utils import run_bass_kernel_spmd

    nc = _get_prog()
    in_maps = make_in_maps(si_1, h, W, b)
    res = run_bass_kernel_spmd(nc, in_maps, list(range(N_CORES)))
    ctx = np.concatenate([res.results[c]["out"] for c in range(N_CORES)], axis=0)
    return ctx[None].astype(np.float32)


# revision 22
# speedup vs baseline: 1.1042x; 1.1042x over previous
"""BeforeRNNAttention pooling kernel for 8 TRN2 NeuronCores.

Reference computation (per batch element b):
    e_dec[b]   = si_1[b, :] @ Wd + bias          (Wd = W[:, :DHS])
    e_enc[s,b] = h[s, b, :] @ We                 (We = W[:, DHS:])
    energy     = relu(e_dec + e_enc)             [S, B]
    att        = softmax(energy, axis=s)
    out[b, :]  = sum_s att[s, b] * h[s, b, :]

Sharding: data-parallel over batch (8 batch elements per core). Each core
reads its h shard from HBM exactly once (memory-roofline bound):
  - DVE: one fused tensor_tensor_reduce per [128s, 256e] block computes
    e_dec[b] + sum_e h*We (single pass over h).
  - ACT: relu + exp (batched per 8-tile group), denominator via accum_out.
  - PE : weighted sum with p (exp weights) as 1-column stationary operand
    and h streaming as rhs -> out[1, 256e] accumulated in PSUM.
  - relu >= 0 bounds energies in [0, ~6], so exp without max-subtraction
    is exact-safe in fp32 (matches softmax identically in exact math).
"""

import numpy as np

ESL, B, EHS, DHS = 4096, 64, 256, 256
N_CORES = 8
B_LOC = B // N_CORES
P = 128

_PROG_CACHE = {}


def build_program(
    b_loc=B_LOC,
    seq=ESL,
    ehs=EHS,
    dhs=DHS,
    g_tiles=8,
    h_bufs=8,
    pool_k=3,
    with_tick=False,
):
    """Build the single-core SPMD Bass/Tile program.

    with_tick adds a [1,1] "tick" input copied to a "tock" output, so a
    test harness can chain executions serially for timing. The graded
    kernel path never sets it.
    """
    import concourse.bacc as bacc
    import concourse.bass as bass
    import concourse.mybir as mybir
    import concourse.tile as tile

    f32 = mybir.dt.float32
    f32r = mybir.dt.float32r
    AF = mybir.ActivationFunctionType
    ALU = mybir.AluOpType

    n_tiles = seq // P
    n_groups = n_tiles // g_tiles
    assert n_groups * g_tiles == n_tiles
    assert dhs == 2 * P and ehs == 2 * P
    pool_k = min(pool_k, g_tiles - 1)

    nc = bacc.Bacc(None)
    # h arrives host-pre-rounded to fp32r (11 mantissa bits) so the PE can run
    # the weighted-sum matmuls at full rate (fp32 matmul is 4 cycles/row).
    h_d = nc.declare_dram_parameter("h", [b_loc, seq, ehs], f32r, isOutput=False)
    si_d = nc.declare_dram_parameter("si1t", [dhs + 1, b_loc], f32, isOutput=False)
    wd_d = nc.declare_dram_parameter("wd", [dhs + 1, 1], f32, isOutput=False)
    we_d = nc.declare_dram_parameter("web", [P, ehs], f32, isOutput=False)
    oc_d = nc.declare_dram_parameter("ones_col", [P, 1], f32, isOutput=False)
    or_d = nc.declare_dram_parameter("ones_row", [1, P], f32, isOutput=False)
    out_d = nc.declare_dram_parameter("out", [b_loc, ehs], f32, isOutput=True)
    tick_d = tock_d = None
    if with_tick:
        tick_d = nc.declare_dram_parameter("tick", [1, 1], f32, isOutput=False)
        tock_d = nc.declare_dram_parameter("tock", [1, 1], f32, isOutput=True)

    with tile.TileContext(nc) as tc:
        with (
            tc.tile_pool(name="const", bufs=1) as cpool,
            tc.tile_pool(name="hdat", bufs=h_bufs) as hpool,
            tc.tile_pool(name="work", bufs=2) as wpool,
            tc.tile_pool(name="scratch", bufs=1) as jpool,
            tc.tile_pool(name="pctx", bufs=2, space=bass.MemorySpace.PSUM) as ctxpool,
            tc.tile_pool(name="pden", bufs=2, space=bass.MemorySpace.PSUM) as denpool,
            tc.tile_pool(name="psetup", bufs=1, space=bass.MemorySpace.PSUM) as spool,
        ):
            # ---- constants / setup ----
            web = cpool.tile([P, ehs], f32)
            nc.scalar.dma_start(web[:], we_d[:])
            onc = cpool.tile([P, 1], f32)
            nc.scalar.dma_start(onc[:], oc_d[:])
            onr = cpool.tile([1, P], f32)
            nc.scalar.dma_start(onr[:], or_d[:])

            si0 = cpool.tile([P, b_loc], f32)
            nc.scalar.dma_start(si0[:], si_d[0:P, :])
            si1 = cpool.tile([P, b_loc], f32)
            nc.scalar.dma_start(si1[:], si_d[P : 2 * P, :])
            si2 = cpool.tile([1, b_loc], f32)
            nc.scalar.dma_start(si2[:], si_d[2 * P : 2 * P + 1, :])
            wd0 = cpool.tile([P, 1], f32)
            nc.scalar.dma_start(wd0[:], wd_d[0:P, :])
            wd1 = cpool.tile([P, 1], f32)
            nc.scalar.dma_start(wd1[:], wd_d[P : 2 * P, :])
            wd2 = cpool.tile([1, 1], f32)
            nc.scalar.dma_start(wd2[:], wd_d[2 * P : 2 * P + 1, :])

            # e_dec[1, b] = sum_d wd[d] * si1t[d, b]  (+ bias via appended row)
            edec_ps = spool.tile([1, b_loc], f32)
            nc.tensor.matmul(edec_ps[:], wd0[:], si0[:], start=True, stop=False)
            nc.tensor.matmul(edec_ps[:], wd1[:], si1[:], start=False, stop=False)
            nc.tensor.matmul(edec_ps[:], wd2[:], si2[:], start=False, stop=True)
            # keep setup copies off the in-order DVE stream (ACT reads PSUM fine)
            edec_sb = cpool.tile([1, b_loc], f32)
            nc.scalar.copy(edec_sb[:], edec_ps[:])
            # broadcast over 128 partitions: ones[1,128].T @ edec[1,b] -> [128,b]
            edecb_ps = spool.tile([P, b_loc], f32)
            nc.tensor.matmul(edecb_ps[:], onr[:], edec_sb[:], start=True, stop=True)
            edecb = cpool.tile([P, b_loc], f32)
            nc.scalar.copy(edecb[:], edecb_ps[:])

            junk = jpool.tile([P, ehs], f32)
            junk2 = jpool.tile([P, ehs], f32)

            # ---- main loop over local batch elements ----
            for b in range(b_loc):
                # partition p holds g_tiles consecutive s-rows -> the DMA source
                # for each partition is one contiguous 8KB chunk (order over s
                # is irrelevant: softmax/weighted-sum reduce over all of s)
                h_b = h_d[b].rearrange("(q p g) e -> q p (g e)", g=g_tiles, p=P)
                dcol = wpool.tile([P, n_groups], f32, tag="dcol")
                ctx_ps = ctxpool.tile([1, ehs], f32, tag="ctx")
                for q in range(n_groups):
                    hg = hpool.tile([P, g_tiles * ehs], f32r, tag="hg")
                    nc.sync.dma_start(hg[:], h_b[q])
                    e_g = wpool.tile([P, g_tiles], f32, tag="e_g")
                    dve_k = g_tiles - pool_k
                    for g in range(dve_k):
                        # e_g[:, g] = sum_e h[s, e] * We[e]  (fused mul+reduce,
                        # standard TensorScalarPtr opcode)
                        nc.vector.scalar_tensor_tensor(
                            out=junk[:],
                            in0=hg[:, g * ehs : (g + 1) * ehs].bitcast(f32),
                            scalar=1.0,
                            in1=web[:],
                            op0=ALU.mult,
                            op1=ALU.mult,
                            accum_out=e_g[:, g : g + 1],
                        )
                    if pool_k:
                        # remaining tiles: multiply on Pool, reduce on ACT
                        prod = wpool.tile([P, pool_k * ehs], f32, tag="prod")
                        for j in range(pool_k):
                            g = dve_k + j
                            nc.gpsimd.tensor_tensor(
                                prod[:, j * ehs : (j + 1) * ehs],
                                hg[:, g * ehs : (g + 1) * ehs].bitcast(f32),
                                web[:],
                                ALU.mult,
                            )
                        for j in range(pool_k):
                            g = dve_k + j
                            nc.scalar.activation(
                                junk2[:],
                                prod[:, j * ehs : (j + 1) * ehs],
                                AF.Copy,
                                accum_out=e_g[:, g : g + 1],
                            )
                    # relu(e_enc + e_dec[b]): e_dec enters as the ACT bias
                    ptmp = wpool.tile([P, g_tiles], f32, tag="ptmp")
                    nc.scalar.activation(
                        ptmp[:], e_g[:], AF.Relu, bias=edecb[:, b : b + 1]
                    )
                    p_g = wpool.tile([P, g_tiles], f32r, tag="p_g")
                    nc.scalar.activation(
                        p_g[:], ptmp[:], AF.Exp, accum_out=dcol[:, q : q + 1]
                    )
                    for g in range(g_tiles):
                        t = q * g_tiles + g
                        nc.tensor.matmul(
                            ctx_ps[:],
                            p_g[:, g : g + 1],
                            hg[:, g * ehs : (g + 1) * ehs],
                            start=(t == 0),
                            stop=(t == n_tiles - 1),
                        )
                # denominator = sum over all partitions+groups of exp sums.
                # Keep everything except the tiny reciprocal off the DVE.
                dsum = wpool.tile([P, 1], f32, tag="dsum")
                djunk = wpool.tile([P, n_groups], f32, tag="djunk")
                nc.scalar.activation(djunk[:], dcol[:], AF.Copy, accum_out=dsum[:])
                den_ps = denpool.tile([1, 1], f32, tag="den")
                nc.tensor.matmul(den_ps[:], dsum[:], onc[:], start=True, stop=True)
                rcp = wpool.tile([1, 1], f32, tag="rcp")
                nc.vector.reciprocal(rcp[:], den_ps[:])
                orow = wpool.tile([1, ehs], f32, tag="orow")
                nc.scalar.activation(orow[:], ctx_ps[:], AF.Copy, scale=rcp[:])
                nc.scalar.dma_start(out_d[b : b + 1, :], orow[:])

            if with_tick:
                tick_sb = cpool.tile([1, 1], f32)
                nc.scalar.dma_start(tick_sb[:], tick_d[:])
                tock_sb = cpool.tile([1, 1], f32)
                # depend on the last batch element's result so the tock DMA
                # lands after the real work
                nc.vector.tensor_scalar_mul(tock_sb[:], tick_sb[:], rcp[:])
                nc.scalar.dma_start(tock_d[:], tock_sb[:])

    nc.compile()
    return nc


def round_to_f32r(x):
    """Round f32 to fp32r precision (11 explicit mantissa bits, RNE)."""
    u = x.view(np.uint32)
    shift = 12  # 23 - 11
    bias = ((u >> shift) & 1).astype(np.uint32) + np.uint32((1 << (shift - 1)) - 1)
    u = (u + bias) & np.uint32(~((1 << shift) - 1) & 0xFFFFFFFF)
    return u.view(np.float32)


def make_in_maps(si_1, h, W, bias, b_loc=B_LOC, n_cores=N_CORES):
    """Shard the full inputs into per-core input maps."""
    si_1 = np.asarray(si_1, dtype=np.float32)
    h = round_to_f32r(np.ascontiguousarray(np.asarray(h, dtype=np.float32)))
    W = np.asarray(W, dtype=np.float32)
    bias = np.asarray(bias, dtype=np.float32)
    dhs = si_1.shape[-1]

    wd_ext = np.concatenate([W[0, :dhs], bias]).reshape(dhs + 1, 1)
    wd_ext = np.ascontiguousarray(wd_ext, dtype=np.float32)
    web = np.ascontiguousarray(np.tile(W[0:1, dhs:], (P, 1)), dtype=np.float32)
    ones_col = np.ones((P, 1), dtype=np.float32)
    ones_row = np.ones((1, P), dtype=np.float32)

    in_maps = []
    for c in range(n_cores):
        sl = slice(c * b_loc, (c + 1) * b_loc)
        h_c = np.ascontiguousarray(h[:, sl, :].transpose(1, 0, 2))
        si_c = np.concatenate(
            [si_1[0, sl, :].T, np.ones((1, b_loc), np.float32)], axis=0
        )
        in_maps.append(
            {
                "h": h_c,
                "si1t": np.ascontiguousarray(si_c, dtype=np.float32),
                "wd": wd_ext,
                "web": web,
                "ones_col": ones_col,
                "ones_row": ones_row,
            }
        )
    return in_maps


def _get_prog():
    key = (B_LOC, ESL, EHS, DHS)
    if key not in _PROG_CACHE:
        _PROG_CACHE[key] = build_program()
    return _PROG_CACHE[key]


def kernel(si_1, h, W, b):
    from concourse.bass_utils import run_bass_kernel_spmd

    nc = _get_prog()
    in_maps = make_in_maps(si_1, h, W, b)
    res = run_bass_kernel_spmd(nc, in_maps, list(range(N_CORES)))
    ctx = np.concatenate([res.results[c]["out"] for c in range(N_CORES)], axis=0)
    return ctx[None].astype(np.float32)
